# revision 26
# baseline (speedup 1.0000x reference)
"""Trainium2 Bass kernel for nn_Attention (B=2, N=2048, DIM=2048, H=16, HD=128).

Sharding: 8 cores = 2 batches x 4 head-groups (4 heads each). Each core:
  - QKV projection for its batch + 4 heads (token-partition layout)
  - rmsnorm applies on DVE (SCALE folded into K's rstd so exp scale is 1)
  - rope on DVE at 2x packed rate (channels de-interleaved host-side so
    even/odd rope halves are contiguous blocks; coefficients broadcast
    across heads via stride-0 APs)
  - PE-transpose of Q/K to [hd, n] layout
  - S^T = K^T.T @ Q^T scores (m on partitions), exp on ACT, softmax sums
    via DVE add-tree + one ones-matmul, 1/sum via reciprocal_approx_fast +
    gpsimd partition_broadcast, PV accumulation; O-normalize deferred two
    heads so DVE never head-of-line blocks
  - output projection partials sprinkled into the next chunk's attention
    m-loops so PE never idles while ACT streams exps
Host sums the 4 head-group partials per batch.
"""

import sys

import numpy as np

sys.path.insert(0, "/opt/trn_rl_repo")

import ml_dtypes  # noqa: E402

import concourse.bass as bass  # noqa: E402
import concourse.tile as tile  # noqa: E402
from concourse import bacc  # noqa: E402
from concourse import mybir  # noqa: E402
from concourse.masks import make_identity  # noqa: E402

B, N, DIM, H, HD = 2, 2048, 2048, 16, 128
NCORES = 8
GROUPS = NCORES // B  # 4 head-groups
HPC = H // GROUPS  # 4 heads per core
CPC = HPC * HD  # 512 channels per core
EPS = 1e-5
SCALE = 1.0 / float(np.sqrt(HD))
EXP_OFF = -7.0  # keeps exp(s) in fp16 range (max observed score*scale ~16); cancels in softmax

NT = N // 128  # 16 token tiles
DT = DIM // 128  # 16 contraction tiles
NJ = N // 512  # 4 n-chunks

F32 = mybir.dt.float32
BF16 = mybir.dt.float16  # fp16: 8x finer mantissa than bf16, same PE rate
NPBF16 = np.float16
AF = mybir.ActivationFunctionType


def _emit(tc: "tile.TileContext"):
    nc = tc.nc
    # x pre-tiled host-side: [p, pair, dt, 256] so each pair DMA reads
    # contiguous 8KB per partition
    xP = nc.dram_tensor("xP", [128, NT // 2, DT, 256], BF16, kind="ExternalInput")
    wqkvT = nc.dram_tensor("wqkvT", [DIM, 3 * CPC], BF16, kind="ExternalInput")
    woutT = nc.dram_tensor("woutT", [CPC, DIM], BF16, kind="ExternalInput")
    coef = nc.dram_tensor("coef", [N, 8, HD // 2], BF16, kind="ExternalInput")
    outp = nc.dram_tensor("outp", [N, DIM], F32, kind="ExternalOutput")

    wq_r = wqkvT.rearrange("(dt p) c -> p dt c", p=128)
    wo_r = woutT.rearrange("(h p) d -> p h d", p=128)

    with (
        tc.tile_pool(name="const", bufs=1) as const,
        tc.tile_pool(name="persist", bufs=1) as persist,
    ):
        ident = const.tile([128, 128], BF16)
        make_identity(nc, ident)
        ones_bf = const.tile([128, 1], BF16)
        nc.vector.memset(ones_bf, 1.0)
        eps_sb = const.tile([128, 1], F32)
        nc.vector.memset(eps_sb, EPS)
        epsk_sb = const.tile([128, 1], F32)
        nc.vector.memset(epsk_sb, EPS * HD)  # eps/SCALE^2: folds SCALE into rstd_k
        expoff_sb = const.tile([128, 1], F32)
        nc.vector.memset(expoff_sb, EXP_OFF)

        # persistent activations, split per 512-token chunk
        QT = [persist.tile([128, HPC, 512], BF16, tag=f"QT{j}", name=f"QT{j}") for j in range(NJ)]
        KT = [persist.tile([128, HPC, 512], BF16, tag=f"KT{j}", name=f"KT{j}") for j in range(NJ)]
        V = [persist.tile([128, 4, CPC], BF16, tag=f"V{j}", name=f"V{j}") for j in range(NJ)]
        O = [persist.tile([128, HPC, 512], BF16, tag=f"O{j}", name=f"O{j}") for j in range(NJ)]

        wout_sb = const.tile([128, HPC, DIM], BF16)

        # ---------------- phase 1: QKV + rmsnorm + rope + transpose ------
        with (
            tc.tile_pool(name="wq", bufs=1) as wqp,
            tc.tile_pool(name="xs", bufs=1) as xsp,
            tc.tile_pool(name="cf", bufs=1) as cfp,
            tc.tile_pool(name="qn", bufs=2) as qnp,
            tc.tile_pool(name="qr", bufs=4) as qrp,
            tc.tile_pool(name="scr", bufs=2) as scrp,
            tc.tile_pool(name="qkv_ps", bufs=2, space="PSUM") as qkvps,
            tc.tile_pool(name="tr_ps", bufs=2, space="PSUM") as trps,
        ):
            # weights: one resident tile; fine chunks first for fast start,
            # alternating between the sync and scalar DMA queues
            wq_sb = wqp.tile([128, DT, 3 * CPC], BF16)
            wslices = [(0, 1), (1, 2), (2, 3), (3, 4)] + [
                (2 * k, 2 * k + 2) for k in range(2, 6)
            ]
            for k, (lo, hi) in enumerate(wslices):
                qeng = nc.sync if k % 2 == 0 else nc.scalar
                qeng.dma_start(out=wq_sb[:, lo:hi, :], in_=wq_r[:, lo:hi, :])

            xtb = [None] * (NT // 2)

            def load_x(pair):
                xb = xsp.tile([128, DT, 256], BF16, tag=f"x{pair % 4}", name=f"x{pair}")
                if pair == 0:
                    nc.gpsimd.dma_start(out=xb[:, 0:4], in_=xP[:, 0, 0:4])
                    nc.gpsimd.dma_start(out=xb[:, 4:], in_=xP[:, 0, 4:])
                else:
                    nc.gpsimd.dma_start(out=xb, in_=xP[:, pair])
                xtb[pair] = xb

            cfs = [None] * NT

            def load_cf(i):
                cf = cfp.tile([128, 8, HD // 2], BF16, tag=f"cf{i % 3}", name=f"cf{i}")
                nc.gpsimd.dma_start(out=cf, in_=coef[i * 128 : (i + 1) * 128])
                cfs[i] = cf

            load_x(0)
            load_cf(0)
            load_cf(1)
            # last two weight pairs ride the gpsimd queue behind the
            # phase-start-critical x/coef transfers
            nc.gpsimd.dma_start(out=wq_sb[:, 12:14, :], in_=wq_r[:, 12:14, :])
            nc.gpsimd.dma_start(out=wq_sb[:, 14:16, :], in_=wq_r[:, 14:16, :])
            load_x(1)
            load_x(2)

            def transposes(i):
                qr = qr_tiles[i % 4]
                for qk in range(2):
                    trp = trps.tile([128, CPC], BF16)
                    for h in range(HPC):
                        hsl = slice(h * HD, (h + 1) * HD)
                        nc.tensor.transpose(trp[:, hsl], qr[:, qk, h, :], ident)
                    tgt = (QT if qk == 0 else KT)[i // 4]
                    dst = tgt[:, :, (i % 4) * 128 : (i % 4 + 1) * 128]
                    nc.vector.tensor_copy(
                        out=dst,
                        in_=trp.rearrange("p (h n) -> p h n", h=HPC),
                    )

            qr_tiles = {}
            for i in range(NT):
                ps = qkvps.tile([128, 3, CPC], F32)
                if i % 2 == 0 and i > 0 and i // 2 + 2 < NT // 2:
                    load_x(i // 2 + 2)
                if i + 2 < NT:
                    load_cf(i + 2)
                if 6 <= i < 10:
                    # output-projection weights, late so they never compete
                    # with the phase-1-critical transfers
                    nc.scalar.dma_start(
                        out=wout_sb[:, i - 6, :], in_=wo_r[:, i - 6, :]
                    )
                xb = xtb[i // 2]
                xsl = slice((i % 2) * 128, (i % 2 + 1) * 128)
                for d in range(DT):
                    for c in range(3):
                        nc.tensor.matmul(
                            ps[:, c, :],
                            lhsT=xb[:, d, xsl],
                            rhs=wq_sb[:, d, c * CPC : (c + 1) * CPC],
                            start=(d == 0),
                            stop=(d == DT - 1),
                        )

                # transposes lag 3 tiles so the PE never waits on the
                # rmsnorm/rope chain of the tile being transposed
                if i > 2:
                    transposes(i - 3)



                # V straight to SBUF (fp16)
                nc.vector.tensor_copy(out=V[i // 4][:, i % 4, :], in_=ps[:, 2, :])

                # rmsnorm per qk so the PSUM reads retire earlier
                ssq = scrp.tile([128, 8], F32, tag="ssq")
                sq = scrp.tile([128, CPC], BF16, tag="sq")
                rstd = scrp.tile([128, 8], F32, tag="rstd")
                qn = qnp.tile([128, 2, HPC, HD], BF16)
                for qk in range(2):
                    for h in range(HPC):
                        hsl = slice(h * HD, (h + 1) * HD)
                        nc.scalar.activation(
                            out=sq[:, hsl],
                            in_=ps[:, qk, hsl],
                            func=AF.Square,
                            accum_out=ssq[:, qk * HPC + h : qk * HPC + h + 1],
                        )
                    sl4 = slice(qk * 4, qk * 4 + 4)
                    # q: 1/sqrt(ssq/HD + eps); k: 1/sqrt(ssq + eps*HD) = SCALE*rstd
                    nc.scalar.activation(
                        rstd[:, sl4],
                        ssq[:, sl4],
                        AF.Sqrt,
                        bias=eps_sb if qk == 0 else epsk_sb,
                        scale=1.0 / HD if qk == 0 else 1.0,
                    )
                    nc.vector.reciprocal(rstd[:, sl4], rstd[:, sl4])
                    for h in range(HPC):
                        nc.vector.tensor_scalar_mul(
                            out=qn[:, qk, h, :],
                            in0=ps[:, qk, h * HD : (h + 1) * HD],
                            scalar1=rstd[:, qk * HPC + h : qk * HPC + h + 1],
                        )

                # rope (gammas folded into coefficients host-side).
                # channels de-interleaved: head layout [even(64) | odd(64)].
                cf = cfs[i]
                qr = qrp.tile([128, 2, HPC, HD], BF16)
                qr_tiles[i % 4] = qr
                ta = scrp.tile([128, HPC, HD // 2], BF16, tag="ta")
                tb = scrp.tile([128, HPC, HD // 2], BF16, tag="tb")
                for qk in range(2):
                    base = qk * 4
                    src = qn[:, qk].rearrange("p h (u e) -> p u h e", u=2)
                    x0 = src[:, 0]  # [128, HPC, 64]
                    x1 = src[:, 1]
                    rot = qr[:, qk].rearrange("p h (u e) -> p u h e", u=2)

                    def cfb(a):
                        return cf[:, a][:, None, :].to_broadcast(
                            [128, HPC, HD // 2]
                        )

                    nc.vector.tensor_mul(ta, x0, cfb(base + 0))
                    nc.vector.tensor_mul(tb, x1, cfb(base + 1))
                    nc.vector.tensor_sub(rot[:, 0], ta, tb)
                    nc.vector.tensor_mul(ta, x0, cfb(base + 2))
                    nc.vector.tensor_mul(tb, x1, cfb(base + 3))
                    nc.vector.tensor_add(rot[:, 1], ta, tb)

            transposes(NT - 3)
            transposes(NT - 2)
            transposes(NT - 1)

        # ------------- phase 2+3: attention + output projection ----------
        with (
            tc.tile_pool(name="s_ps", bufs=4, space="PSUM") as sps,
            tc.tile_pool(name="o_ps", bufs=2, space="PSUM") as ops_,
            tc.tile_pool(name="op_ps", bufs=2, space="PSUM") as opps,
            tc.tile_pool(name="es", bufs=3) as esp,
            tc.tile_pool(name="tr2", bufs=3) as trsp,
            tc.tile_pool(name="invb", bufs=3) as invbp,
            tc.tile_pool(name="ob", bufs=3) as obp,
        ):
            def attention(j, h, sprinkle):
                # sprinkle: list of closures (outproj steps) to interleave
                o_ps = ops_.tile([128, 512], F32)
                es_all = esp.tile([128, NT, 512], BF16)
                t = trsp.tile([128, 4, 512], BF16)

                def pop(k):
                    for _ in range(k):
                        if sprinkle:
                            st = sprinkle.pop(0)
                            if st is not None:
                                st()

                def pv(m):
                    nc.tensor.matmul(
                        o_ps,
                        lhsT=V[m // 4][:, m % 4, h * HD : (h + 1) * HD],
                        rhs=es_all[:, m, :],
                        start=(m == 0),
                        stop=(m == NT - 1),
                    )

                for m in range(NT):
                    s1 = sps.tile([128, 512], F32, tag="s1")
                    nc.tensor.matmul(
                        s1,
                        lhsT=KT[m // 4][:, h, (m % 4) * 128 : (m % 4 + 1) * 128],
                        rhs=QT[j][:, h, :],
                        start=True,
                        stop=True,
                    )
                    nc.scalar.activation(
                        es_all[:, m, :], s1, AF.Exp, bias=expoff_sb
                    )
                    pop(1)
                    if m >= 4:
                        pv(m - 4)
                        pop(1)
                    if m == 11:
                        # early half of the denominator tree
                        nc.vector.tensor_add(t, es_all[:, 0:4, :], es_all[:, 4:8, :])
                for m in range(12, 16):
                    pv(m)
                    pop(1)
                return (j, h, o_ps, es_all, t)

            def norm(j, h, o_ps, es_all, t):
                # denominators: finish the DVE tree over the 16 m-tiles
                nc.vector.tensor_add(t, t, es_all[:, 8:12, :])
                nc.vector.tensor_add(t, t, es_all[:, 12:16, :])
                nc.vector.tensor_add(t[:, 0:2], t[:, 0:2], t[:, 2:4])
                nc.vector.tensor_add(t[:, 0], t[:, 0], t[:, 1])
                den_ps = sps.tile([128, 512], F32, tag="s1")
                nc.tensor.matmul(
                    den_ps[0:1, :], lhsT=ones_bf, rhs=t[:, 0], start=True, stop=True
                )
                inv = invbp.tile([128, 512], F32, tag="inv")
                nc.vector.reciprocal_approx_fast(out=inv[0:1, :], in_=den_ps[0:1, :])
                invb = invbp.tile([128, 512], F32, tag="invb")
                nc.gpsimd.partition_broadcast(invb, inv[0:1, :])
                return (j, h, o_ps, invb)

            def mul_O(j, h, o_ps, invb):
                nc.vector.tensor_mul(O[j][:, h, :], o_ps, invb)

            def outproj_steps(j, act_copy=False):
                # closures: per (it,dch) 4 accumulating matmuls, an SBUF
                # bounce copy, then the output DMA
                steps = []
                for it in range(4):
                    nsl = slice((4 * j + it) * 128, (4 * j + it + 1) * 128)
                    for dch in range(4):
                        dsl = slice(dch * 512, (dch + 1) * 512)
                        op_ps = opps.tile([128, 512], F32, tag="op", name=f"op{j}_{it}_{dch}")

                        def mk_mm(op_ps=op_ps, it=it, dsl=dsl, h=0):
                            return lambda: nc.tensor.matmul(
                                op_ps,
                                lhsT=O[j][:, h, it * 128 : (it + 1) * 128],
                                rhs=wout_sb[:, h, dsl],
                                start=(h == 0),
                                stop=(h == HPC - 1),
                            )

                        for h in range(HPC):
                            steps.append(mk_mm(h=h))
                        qeng = nc.sync if (it + dch) % 2 == 0 else nc.gpsimd
                        ob = obp.tile([128, 512], F32, tag="ob", name=f"ob{j}_{it}_{dch}")
                        if act_copy:
                            steps.append(
                                lambda op_ps=op_ps, ob=ob: nc.scalar.copy(ob, op_ps)
                            )
                        else:
                            steps.append(
                                lambda op_ps=op_ps, ob=ob: nc.vector.tensor_copy(
                                    out=ob, in_=op_ps
                                )
                            )
                        steps.append(
                            lambda ob=ob, nsl=nsl, dsl=dsl, qeng=qeng: qeng.dma_start(
                                out=outp[nsl, dsl], in_=ob
                            )
                        )
                return steps

            pend_front = None
            pend_mul = None
            sprinkle = []
            for j in range(NJ):
                for h in range(HPC):
                    if pend_mul is not None:
                        mul_O(*pend_mul)
                    cur = attention(j, h, sprinkle)
                    if pend_front is not None:
                        pend_mul = norm(*pend_front)
                    else:
                        pend_mul = None
                    pend_front = cur
                    if h == 0 and j > 0:
                        for st in sprinkle:  # drain leftovers before swapping
                            if st is not None:
                                st()
                        sprinkle = outproj_steps(j - 1)
            pm = norm(*pend_front)
            if pend_mul is not None:
                mul_O(*pend_mul)
            mul_O(*pm)
            for st in sprinkle:
                if st is not None:
                    st()
            for st in outproj_steps(NJ - 1):
                st()


_NC = None


def _get_nc():
    global _NC
    if _NC is None:
        nc = bacc.Bacc()
        with tile.TileContext(nc) as tc:
            _emit(tc)
        if not nc.is_finalized():
            nc.finalize()
        _NC = nc
    return _NC


def _prep_core(x, Wqkv, q_gamma, k_gamma, Wout, cos, sin, b, hg):
    # de-interleave rope pairs within each head: [e0..e63 | o0..o63]
    perm = np.concatenate([np.arange(0, HD, 2), np.arange(1, HD, 2)])
    hsl = slice(hg * CPC, (hg + 1) * CPC)
    Wq = Wqkv[0 * H * HD : 1 * H * HD][hsl].reshape(HPC, HD, DIM)[:, perm].reshape(CPC, DIM)
    Wk = Wqkv[1 * H * HD : 2 * H * HD][hsl].reshape(HPC, HD, DIM)[:, perm].reshape(CPC, DIM)
    Wv = Wqkv[2 * H * HD : 3 * H * HD][hsl]
    wqkvT = np.ascontiguousarray(np.concatenate([Wq, Wk, Wv], 0).T)
    woutT = np.ascontiguousarray(Wout[:, hsl].T)

    # x tiled to [p, pair, dt, 256] (contiguous per-partition DMA reads)
    xT = x[b].T.astype(NPBF16)  # [DIM, N]
    xP = np.ascontiguousarray(
        xT.reshape(DT, 128, NT // 2, 256).transpose(1, 2, 0, 3)
    )

    qe, qo = q_gamma[0::2], q_gamma[1::2]
    ke, ko = k_gamma[0::2], k_gamma[1::2]
    cb, sb = cos[b], sin[b]  # [N, 64]
    coef = np.stack(
        [
            cb * qe, sb * qo, sb * qe, cb * qo,
            cb * ke, sb * ko, sb * ke, cb * ko,
        ],
        axis=1,
    ).astype(np.float16)  # [N, 8, 64]
    return {
        "xP": xP,
        "wqkvT": wqkvT.astype(NPBF16),
        "woutT": woutT.astype(NPBF16),
        "coef": np.ascontiguousarray(coef),
    }


def prep_in_maps(x, Wqkv, q_gamma, k_gamma, Wout, freqs):
    x = np.asarray(x, np.float32)
    Wqkv = np.asarray(Wqkv, np.float32)
    Wout = np.asarray(Wout, np.float32)
    q_gamma = np.asarray(q_gamma, np.float32)
    k_gamma = np.asarray(k_gamma, np.float32)
    freqs = np.asarray(freqs, np.float32)
    cos = freqs[..., 0]
    sin = freqs[..., 1]
    return [
        _prep_core(x, Wqkv, q_gamma, k_gamma, Wout, cos, sin, c // GROUPS, c % GROUPS)
        for c in range(NCORES)
    ]


def gather(parts):
    out = np.empty((B, N, DIM), np.float32)
    for b in range(B):
        acc = parts[b * GROUPS].astype(np.float32)
        for g in range(1, GROUPS):
            acc = acc + parts[b * GROUPS + g]
        out[b] = acc
    return out


def kernel(x, Wqkv, q_gamma, k_gamma, Wout, freqs):
    from concourse.bass_utils import run_bass_kernel_spmd

    nc = _get_nc()
    in_maps = prep_in_maps(x, Wqkv, q_gamma, k_gamma, Wout, freqs)
    res = run_bass_kernel_spmd(nc, in_maps, list(range(NCORES)))
    parts = [res.results[c]["outp"] for c in range(NCORES)]
    return gather(parts)


# revision 27
# speedup vs baseline: 1.1836x; 1.1836x over previous
"""Trainium2 Bass kernel for nn_Attention (B=2, N=2048, DIM=2048, H=16, HD=128).

Sharding: 8 cores = 2 batches x 4 head-groups (4 heads each). Each core:
  - QKV projection for its batch + 4 heads (token-partition layout)
  - rmsnorm applies on DVE (SCALE folded into K's rstd so exp scale is 1)
  - rope on DVE at 2x packed rate (channels de-interleaved host-side so
    even/odd rope halves are contiguous blocks; coefficients broadcast
    across heads via stride-0 APs)
  - PE-transpose of Q/K to [hd, n] layout
  - S^T = K^T.T @ Q^T scores (m on partitions), exp on ACT, softmax sums
    via DVE add-tree + one ones-matmul, 1/sum via reciprocal_approx_fast +
    gpsimd partition_broadcast, PV accumulation; O-normalize deferred two
    heads so DVE never head-of-line blocks
  - output projection partials sprinkled into the next chunk's attention
    m-loops so PE never idles while ACT streams exps
Host sums the 4 head-group partials per batch.
"""

import sys

import numpy as np

sys.path.insert(0, "/opt/trn_rl_repo")

import ml_dtypes  # noqa: E402

import concourse.bass as bass  # noqa: E402
import concourse.tile as tile  # noqa: E402
from concourse import bacc  # noqa: E402
from concourse import mybir  # noqa: E402
from concourse.masks import make_identity  # noqa: E402

B, N, DIM, H, HD = 2, 2048, 2048, 16, 128
NCORES = 8
GROUPS = NCORES // B  # 4 head-groups
HPC = H // GROUPS  # 4 heads per core
CPC = HPC * HD  # 512 channels per core
EPS = 1e-5
SCALE = 1.0 / float(np.sqrt(HD))
EXP_OFF = -7.0  # keeps exp(s) in fp16 range (max observed score*scale ~16); cancels in softmax

NT = N // 128  # 16 token tiles
DT = DIM // 128  # 16 contraction tiles
NJ = N // 512  # 4 n-chunks

F32 = mybir.dt.float32
BF16 = mybir.dt.float16  # fp16: 8x finer mantissa than bf16, same PE rate
NPBF16 = np.float16
AF = mybir.ActivationFunctionType


def _emit(tc: "tile.TileContext"):
    nc = tc.nc
    # x pre-tiled host-side: [p, pair, dt, 256] so each pair DMA reads
    # contiguous 8KB per partition
    xP = nc.dram_tensor("xP", [128, NT // 2, DT, 256], BF16, kind="ExternalInput")
    wqkvT = nc.dram_tensor("wqkvT", [DIM, 3 * CPC], BF16, kind="ExternalInput")
    woutT = nc.dram_tensor("woutT", [CPC, DIM], BF16, kind="ExternalInput")
    coef = nc.dram_tensor("coef", [N, 8, HD // 2], BF16, kind="ExternalInput")
    outp = nc.dram_tensor("outp", [N, DIM], F32, kind="ExternalOutput")

    wq_r = wqkvT.rearrange("(dt p) c -> p dt c", p=128)
    wo_r = woutT.rearrange("(h p) d -> p h d", p=128)

    with (
        tc.tile_pool(name="const", bufs=1) as const,
        tc.tile_pool(name="persist", bufs=1) as persist,
    ):
        ident = const.tile([128, 128], BF16)
        make_identity(nc, ident)
        ones_bf = const.tile([128, 1], BF16)
        nc.vector.memset(ones_bf, 1.0)
        eps_sb = const.tile([128, 1], F32)
        nc.vector.memset(eps_sb, EPS)
        epsk_sb = const.tile([128, 1], F32)
        nc.vector.memset(epsk_sb, EPS * HD)  # eps/SCALE^2: folds SCALE into rstd_k
        expoff_sb = const.tile([128, 1], F32)
        nc.vector.memset(expoff_sb, EXP_OFF)

        # persistent activations, split per 512-token chunk
        QT = [persist.tile([128, HPC, 512], BF16, tag=f"QT{j}", name=f"QT{j}") for j in range(NJ)]
        KT = [persist.tile([128, HPC, 512], BF16, tag=f"KT{j}", name=f"KT{j}") for j in range(NJ)]
        V = [persist.tile([128, 4, CPC], BF16, tag=f"V{j}", name=f"V{j}") for j in range(NJ)]
        O = [persist.tile([128, HPC, 512], BF16, tag=f"O{j}", name=f"O{j}") for j in range(NJ)]

        wout_sb = const.tile([128, HPC, DIM], BF16)

        # ---------------- phase 1: QKV + rmsnorm + rope + transpose ------
        with (
            tc.tile_pool(name="wq", bufs=1) as wqp,
            tc.tile_pool(name="xs", bufs=1) as xsp,
            tc.tile_pool(name="cf", bufs=1) as cfp,
            tc.tile_pool(name="qn", bufs=2) as qnp,
            tc.tile_pool(name="qr", bufs=4) as qrp,
            tc.tile_pool(name="scr", bufs=2) as scrp,
            tc.tile_pool(name="qkv_ps", bufs=2, space="PSUM") as qkvps,
            tc.tile_pool(name="tr_ps", bufs=2, space="PSUM") as trps,
        ):
            # weights: one resident tile; fine chunks first for fast start,
            # alternating between the sync and scalar DMA queues
            wq_sb = wqp.tile([128, DT, 3 * CPC], BF16)
            wslices = [(0, 1), (1, 2), (2, 3), (3, 4)] + [
                (2 * k, 2 * k + 2) for k in range(2, 6)
            ]
            for k, (lo, hi) in enumerate(wslices):
                qeng = nc.sync if k % 2 == 0 else nc.scalar
                qeng.dma_start(out=wq_sb[:, lo:hi, :], in_=wq_r[:, lo:hi, :])

            xtb = [None] * (NT // 2)

            def load_x(pair):
                xb = xsp.tile([128, DT, 256], BF16, tag=f"x{pair % 4}", name=f"x{pair}")
                if pair == 0:
                    nc.gpsimd.dma_start(out=xb[:, 0:4], in_=xP[:, 0, 0:4])
                    nc.gpsimd.dma_start(out=xb[:, 4:], in_=xP[:, 0, 4:])
                else:
                    nc.gpsimd.dma_start(out=xb, in_=xP[:, pair])
                xtb[pair] = xb

            cfs = [None] * NT

            def load_cf(i):
                cf = cfp.tile([128, 8, HD // 2], BF16, tag=f"cf{i % 3}", name=f"cf{i}")
                nc.gpsimd.dma_start(out=cf, in_=coef[i * 128 : (i + 1) * 128])
                cfs[i] = cf

            load_x(0)
            load_cf(0)
            load_cf(1)
            # last two weight pairs ride the gpsimd queue behind the
            # phase-start-critical x/coef transfers
            nc.gpsimd.dma_start(out=wq_sb[:, 12:14, :], in_=wq_r[:, 12:14, :])
            nc.gpsimd.dma_start(out=wq_sb[:, 14:16, :], in_=wq_r[:, 14:16, :])
            load_x(1)
            load_x(2)

            def transposes(i):
                qr = qr_tiles[i % 4]
                for qk in range(2):
                    trp = trps.tile([128, CPC], BF16)
                    for h in range(HPC):
                        hsl = slice(h * HD, (h + 1) * HD)
                        nc.tensor.transpose(trp[:, hsl], qr[:, qk, h, :], ident)
                    tgt = (QT if qk == 0 else KT)[i // 4]
                    dst = tgt[:, :, (i % 4) * 128 : (i % 4 + 1) * 128]
                    nc.vector.tensor_copy(
                        out=dst,
                        in_=trp.rearrange("p (h n) -> p h n", h=HPC),
                    )

            qr_tiles = {}
            for i in range(NT):
                ps = qkvps.tile([128, 3, CPC], F32)
                if i % 2 == 0 and i > 0 and i // 2 + 2 < NT // 2:
                    load_x(i // 2 + 2)
                if i + 2 < NT:
                    load_cf(i + 2)
                if 6 <= i < 10:
                    # output-projection weights, late so they never compete
                    # with the phase-1-critical transfers
                    nc.scalar.dma_start(
                        out=wout_sb[:, i - 6, :], in_=wo_r[:, i - 6, :]
                    )
                xb = xtb[i // 2]
                xsl = slice((i % 2) * 128, (i % 2 + 1) * 128)
                for d in range(DT):
                    for c in range(3):
                        nc.tensor.matmul(
                            ps[:, c, :],
                            lhsT=xb[:, d, xsl],
                            rhs=wq_sb[:, d, c * CPC : (c + 1) * CPC],
                            start=(d == 0),
                            stop=(d == DT - 1),
                        )

                # transposes lag 3 tiles so the PE never waits on the
                # rmsnorm/rope chain of the tile being transposed
                if i > 2:
                    transposes(i - 3)



                # V straight to SBUF (fp16)
                nc.vector.tensor_copy(out=V[i // 4][:, i % 4, :], in_=ps[:, 2, :])

                # rmsnorm: sum of squares per head -> rstd
                ssq = scrp.tile([128, 8], F32, tag="ssq")
                sq = scrp.tile([128, CPC], BF16, tag="sq")
                for qk in range(2):
                    for h in range(HPC):
                        hsl = slice(h * HD, (h + 1) * HD)
                        nc.scalar.activation(
                            out=sq[:, hsl],
                            in_=ps[:, qk, hsl],
                            func=AF.Square,
                            accum_out=ssq[:, qk * HPC + h : qk * HPC + h + 1],
                        )
                rstd = scrp.tile([128, 8], F32, tag="rstd")
                # q: 1/sqrt(ssq/HD + eps); k: 1/sqrt(ssq + eps*HD) = SCALE*rstd
                nc.scalar.activation(
                    rstd[:, 0:4], ssq[:, 0:4], AF.Sqrt, bias=eps_sb, scale=1.0 / HD
                )
                nc.scalar.activation(
                    rstd[:, 4:8], ssq[:, 4:8], AF.Sqrt, bias=epsk_sb, scale=1.0
                )
                nc.vector.reciprocal(rstd, rstd)

                qn = qnp.tile([128, 2, HPC, HD], BF16)
                for qk in range(2):
                    for h in range(HPC):
                        nc.vector.tensor_scalar_mul(
                            out=qn[:, qk, h, :],
                            in0=ps[:, qk, h * HD : (h + 1) * HD],
                            scalar1=rstd[:, qk * HPC + h : qk * HPC + h + 1],
                        )

                # rope (gammas folded into coefficients host-side).
                # channels de-interleaved: head layout [even(64) | odd(64)].
                cf = cfs[i]
                qr = qrp.tile([128, 2, HPC, HD], BF16)
                qr_tiles[i % 4] = qr
                ta = scrp.tile([128, HPC, HD // 2], BF16, tag="ta")
                tb = scrp.tile([128, HPC, HD // 2], BF16, tag="tb")
                for qk in range(2):
                    base = qk * 4
                    src = qn[:, qk].rearrange("p h (u e) -> p u h e", u=2)
                    x0 = src[:, 0]  # [128, HPC, 64]
                    x1 = src[:, 1]
                    rot = qr[:, qk].rearrange("p h (u e) -> p u h e", u=2)

                    def cfb(a):
                        return cf[:, a][:, None, :].to_broadcast(
                            [128, HPC, HD // 2]
                        )

                    nc.vector.tensor_mul(ta, x0, cfb(base + 0))
                    nc.vector.tensor_mul(tb, x1, cfb(base + 1))
                    nc.vector.tensor_sub(rot[:, 0], ta, tb)
                    nc.vector.tensor_mul(ta, x0, cfb(base + 2))
                    nc.vector.tensor_mul(tb, x1, cfb(base + 3))
                    nc.vector.tensor_add(rot[:, 1], ta, tb)

            transposes(NT - 3)
            transposes(NT - 2)
            transposes(NT - 1)

        # ------------- phase 2+3: attention + output projection ----------
        with (
            tc.tile_pool(name="s_ps", bufs=4, space="PSUM") as sps,
            tc.tile_pool(name="o_ps", bufs=2, space="PSUM") as ops_,
            tc.tile_pool(name="op_ps", bufs=2, space="PSUM") as opps,
            tc.tile_pool(name="es", bufs=3) as esp,
            tc.tile_pool(name="tr2", bufs=3) as trsp,
            tc.tile_pool(name="invb", bufs=3) as invbp,
            tc.tile_pool(name="ob", bufs=3) as obp,
        ):
            def attention(j, h, sprinkle):
                # sprinkle: list of closures (outproj steps) to interleave
                o_ps = ops_.tile([128, 512], F32)
                es_all = esp.tile([128, NT, 512], BF16)
                t = trsp.tile([128, 4, 512], BF16)

                def pop(k):
                    for _ in range(k):
                        if sprinkle:
                            st = sprinkle.pop(0)
                            if st is not None:
                                st()

                def pv(m):
                    nc.tensor.matmul(
                        o_ps,
                        lhsT=V[m // 4][:, m % 4, h * HD : (h + 1) * HD],
                        rhs=es_all[:, m, :],
                        start=(m == 0),
                        stop=(m == NT - 1),
                    )

                for m in range(NT):
                    s1 = sps.tile([128, 512], F32, tag="s1")
                    nc.tensor.matmul(
                        s1,
                        lhsT=KT[m // 4][:, h, (m % 4) * 128 : (m % 4 + 1) * 128],
                        rhs=QT[j][:, h, :],
                        start=True,
                        stop=True,
                    )
                    nc.scalar.activation(
                        es_all[:, m, :], s1, AF.Exp, bias=expoff_sb
                    )
                    pop(1)
                    if m >= 4:
                        pv(m - 4)
                        pop(1)
                    if m == 11:
                        # early half of the denominator tree
                        nc.vector.tensor_add(t, es_all[:, 0:4, :], es_all[:, 4:8, :])
                for m in range(12, 16):
                    pv(m)
                    pop(1)
                return (j, h, o_ps, es_all, t)

            def norm(j, h, o_ps, es_all, t):
                # denominators: finish the DVE tree over the 16 m-tiles
                nc.vector.tensor_add(t, t, es_all[:, 8:12, :])
                nc.vector.tensor_add(t, t, es_all[:, 12:16, :])
                nc.vector.tensor_add(t[:, 0:2], t[:, 0:2], t[:, 2:4])
                nc.vector.tensor_add(t[:, 0], t[:, 0], t[:, 1])
                den_ps = sps.tile([128, 512], F32, tag="s1")
                nc.tensor.matmul(
                    den_ps[0:1, :], lhsT=ones_bf, rhs=t[:, 0], start=True, stop=True
                )
                inv = invbp.tile([128, 512], F32, tag="inv")
                nc.vector.reciprocal_approx_fast(out=inv[0:1, :], in_=den_ps[0:1, :])
                invb = invbp.tile([128, 512], F32, tag="invb")
                nc.gpsimd.partition_broadcast(invb, inv[0:1, :])
                return (j, h, o_ps, invb)

            def mul_O(j, h, o_ps, invb):
                nc.vector.tensor_mul(O[j][:, h, :], o_ps, invb)

            def outproj_steps(j, act_copy=False):
                # closures: per (it,dch) 4 accumulating matmuls, an SBUF
                # bounce copy, then the output DMA
                steps = []
                for it in range(4):
                    nsl = slice((4 * j + it) * 128, (4 * j + it + 1) * 128)
                    for dch in range(4):
                        dsl = slice(dch * 512, (dch + 1) * 512)
                        op_ps = opps.tile([128, 512], F32, tag="op", name=f"op{j}_{it}_{dch}")

                        def mk_mm(op_ps=op_ps, it=it, dsl=dsl, h=0):
                            return lambda: nc.tensor.matmul(
                                op_ps,
                                lhsT=O[j][:, h, it * 128 : (it + 1) * 128],
                                rhs=wout_sb[:, h, dsl],
                                start=(h == 0),
                                stop=(h == HPC - 1),
                            )

                        for h in range(HPC):
                            steps.append(mk_mm(h=h))
                        qeng = nc.sync if (it + dch) % 2 == 0 else nc.gpsimd
                        ob = obp.tile([128, 512], F32, tag="ob", name=f"ob{j}_{it}_{dch}")
                        if act_copy:
                            steps.append(
                                lambda op_ps=op_ps, ob=ob: nc.scalar.copy(ob, op_ps)
                            )
                        else:
                            steps.append(
                                lambda op_ps=op_ps, ob=ob: nc.vector.tensor_copy(
                                    out=ob, in_=op_ps
                                )
                            )
                        steps.append(
                            lambda ob=ob, nsl=nsl, dsl=dsl, qeng=qeng: qeng.dma_start(
                                out=outp[nsl, dsl], in_=ob
                            )
                        )
                return steps

            pend_front = None
            pend_mul = None
            sprinkle = []
            for j in range(NJ):
                for h in range(HPC):
                    if pend_mul is not None:
                        mul_O(*pend_mul)
                    cur = attention(j, h, sprinkle)
                    if pend_front is not None:
                        pend_mul = norm(*pend_front)
                    else:
                        pend_mul = None
                    pend_front = cur
                    if h == 0 and j > 0:
                        for st in sprinkle:  # drain leftovers before swapping
                            if st is not None:
                                st()
                        sprinkle = outproj_steps(j - 1)
            pm = norm(*pend_front)
            if pend_mul is not None:
                mul_O(*pend_mul)
            mul_O(*pm)
            for st in sprinkle:
                if st is not None:
                    st()
            for st in outproj_steps(NJ - 1):
                st()


_NC = None


def _get_nc():
    global _NC
    if _NC is None:
        nc = bacc.Bacc()
        with tile.TileContext(nc) as tc:
            _emit(tc)
        if not nc.is_finalized():
            nc.finalize()
        _NC = nc
    return _NC


def _prep_core(x, Wqkv, q_gamma, k_gamma, Wout, cos, sin, b, hg):
    # de-interleave rope pairs within each head: [e0..e63 | o0..o63]
    perm = np.concatenate([np.arange(0, HD, 2), np.arange(1, HD, 2)])
    hsl = slice(hg * CPC, (hg + 1) * CPC)
    Wq = Wqkv[0 * H * HD : 1 * H * HD][hsl].reshape(HPC, HD, DIM)[:, perm].reshape(CPC, DIM)
    Wk = Wqkv[1 * H * HD : 2 * H * HD][hsl].reshape(HPC, HD, DIM)[:, perm].reshape(CPC, DIM)
    Wv = Wqkv[2 * H * HD : 3 * H * HD][hsl]
    wqkvT = np.ascontiguousarray(np.concatenate([Wq, Wk, Wv], 0).T)
    woutT = np.ascontiguousarray(Wout[:, hsl].T)

    # x tiled to [p, pair, dt, 256] (contiguous per-partition DMA reads)
    xT = x[b].T.astype(NPBF16)  # [DIM, N]
    xP = np.ascontiguousarray(
        xT.reshape(DT, 128, NT // 2, 256).transpose(1, 2, 0, 3)
    )

    qe, qo = q_gamma[0::2], q_gamma[1::2]
    ke, ko = k_gamma[0::2], k_gamma[1::2]
    cb, sb = cos[b], sin[b]  # [N, 64]
    coef = np.stack(
        [
            cb * qe, sb * qo, sb * qe, cb * qo,
            cb * ke, sb * ko, sb * ke, cb * ko,
        ],
        axis=1,
    ).astype(np.float16)  # [N, 8, 64]
    return {
        "xP": xP,
        "wqkvT": wqkvT.astype(NPBF16),
        "woutT": woutT.astype(NPBF16),
        "coef": np.ascontiguousarray(coef),
    }


def prep_in_maps(x, Wqkv, q_gamma, k_gamma, Wout, freqs):
    x = np.asarray(x, np.float32)
    Wqkv = np.asarray(Wqkv, np.float32)
    Wout = np.asarray(Wout, np.float32)
    q_gamma = np.asarray(q_gamma, np.float32)
    k_gamma = np.asarray(k_gamma, np.float32)
    freqs = np.asarray(freqs, np.float32)
    cos = freqs[..., 0]
    sin = freqs[..., 1]
    return [
        _prep_core(x, Wqkv, q_gamma, k_gamma, Wout, cos, sin, c // GROUPS, c % GROUPS)
        for c in range(NCORES)
    ]


def gather(parts):
    out = np.empty((B, N, DIM), np.float32)
    for b in range(B):
        acc = parts[b * GROUPS].astype(np.float32)
        for g in range(1, GROUPS):
            acc = acc + parts[b * GROUPS + g]
        out[b] = acc
    return out


def kernel(x, Wqkv, q_gamma, k_gamma, Wout, freqs):
    from concourse.bass_utils import run_bass_kernel_spmd

    nc = _get_nc()
    in_maps = prep_in_maps(x, Wqkv, q_gamma, k_gamma, Wout, freqs)
    res = run_bass_kernel_spmd(nc, in_maps, list(range(NCORES)))
    parts = [res.results[c]["outp"] for c in range(NCORES)]
    return gather(parts)
